# revision 7
# baseline (speedup 1.0000x reference)
"""Trainium2 Bass kernel for ClassificationMPNSimple (message-passing GNN).

Strategy: shard edges across 8 NeuronCores BY TARGET-NODE BLOCK (core c owns
nodes [1250c, 1250(c+1)) and every edge whose target lands there).  Node
features live in a per-core DRAM gather table [10000, 256] bf16 = [nf0 | nf];
per-edge source/target features are fetched with dma_gather(transpose=True)
(feature-major output feeds the matmul pipeline directly).  Aggregation
(segment_sum over targets) is done with one-hot matmuls: edges are host-sorted
by 128-node target block, and per 128-edge tile a host-built one-hot P maps
messages to block node slots, accumulated in PSUM across the block's tiles.
The per-step node update is local to each core; an AllGather of the updated
node features refreshes the gather tables between steps (only needed after
step 1).  Edge/node classifier heads are fused into step 2.

All matmuls run in bf16 with fp32 PSUM accumulation.
"""

import numpy as np
import ml_dtypes

import concourse.bass as bass
import concourse.bacc as bacc
import concourse.tile as tile
import concourse.mybir as mybir
from concourse.bass_utils import run_bass_kernel_spmd

dt = mybir.dt
AF = mybir.ActivationFunctionType

# ---- problem constants (hardcoded; matches setup_inputs shapes) ----
N = 10000          # nodes
E = 200000         # edges
F = 128            # feature dim
NC = 8             # cores
NPC = 1250         # nodes per core
NB = 10            # 128-node blocks per core
P = 128
TOKB = 512         # token batch (PSUM free-dim limit)
GCH = 512          # gather chunk (HW limit: large num_idxs gathers abort)

BF = dt.bfloat16
F32 = dt.float32

_prog_cache = {}


# --------------------------------------------------------------------------
# program builder
# --------------------------------------------------------------------------

def _build_program(Tb, stop_after=4):
    EP = NB * Tb * P          # padded edges per core
    assert EP % TOKB == 0 and EP % GCH == 0 and GCH % TOKB == 0
    NBAT = EP // TOKB
    BLK = Tb * P              # tokens per node block

    nc = bacc.Bacc("TRN2", target_bir_lowering=False, debug=False,
                   enable_asserts=True, num_devices=NC)

    def inp(name, shape, dty):
        return nc.dram_tensor(name, shape, dty, kind="ExternalInput").ap()

    # replicated inputs
    x_fm = inp("x_fm", [P, N], BF)                    # x transposed, bf16
    Wne1 = inp("Wne1", [P, F], BF); bne1 = inp("bne1", [P, 1], F32)
    Wne2 = inp("Wne2", [P, F], BF); bne2r = inp("bne2r", [1, F], BF)
    Wee1 = inp("Wee1", [64, F], BF); bee1 = inp("bee1", [P, 1], F32)
    Wee2 = inp("Wee2", [P, F], BF); bee2 = inp("bee2", [P, 1], F32)
    We1 = inp("We1", [P, 6 * 512], BF)                # [p, kc*512+f] = We1[kc*128+p, f]
    be1 = inp("be1", [P, 4], F32)                     # [p, oc] = be1[oc*128+p]
    We2 = inp("We2", [P, 4 * F], BF)                  # [p, kc*128+f] = We2[kc*128+p, f]
    be2 = inp("be2", [P, 1], F32)
    Wn1 = inp("Wn1", [P, 3 * F], BF)                  # [p, kc*128+f] = Wn1[kc*128+p, f]
    bn1r = inp("bn1r", [1, F], BF)
    Wu = inp("Wu", [P, F], BF)
    bur = inp("bur", [1, F], BF)                      # row bias (step1, free-dim)
    bu = inp("bu", [P, 1], F32)                       # col bias (step2, partition-dim)
    Wnc1 = inp("Wnc1", [P, F], BF); bnc1 = inp("bnc1", [P, 1], F32)
    Wnc2 = inp("Wnc2", [P, 1], BF); bnc2 = inp("bnc2", [1, 1], F32)
    Wec1 = inp("Wec1", [P, F], BF); bec1 = inp("bec1", [P, 1], F32)
    Wec2 = inp("Wec2", [P, 1], BF); bec2 = inp("bec2", [1, 1], F32)

    # per-core inputs
    ea_fm = inp("ea_fm", [64, EP], BF)                # edge_attr transposed (sorted order)
    idx_i = inp("idx_i", [P, EP // 16], dt.int16)     # wrapped + Q7-replicated
    idx_j = inp("idx_j", [P, EP // 16], dt.int16)
    Pm = inp("Pm", [NB, P, BLK], BF)                  # one-hot scatter matrices

    # outputs
    pe_o = nc.dram_tensor("preds_edge", [1, EP], F32, kind="ExternalOutput").ap()
    pn_o = nc.dram_tensor("preds_node", [1, NB * P], F32, kind="ExternalOutput").ap()

    # internal DRAM
    table = nc.dram_tensor("table", [N, 2 * F], BF).ap()
    nf0_rows = nc.dram_tensor("nf0_rows", [N, F], BF).ap()
    ef0_fm = nc.dram_tensor("ef0_fm", [P, EP], BF).ap()
    ef1_fm = nc.dram_tensor("ef1_fm", [P, EP], BF).ap()
    nf1_rows = nc.dram_tensor("nf1_rows", [NB * P, F], BF).ap()
    nf1_all = nc.dram_tensor("nf1_all", [N, F], BF, addr_space="Shared").ap()

    with tile.TileContext(nc) as tc:
        with tc.tile_pool(name="wpool", bufs=1) as wp, \
             tc.tile_pool(name="gath", bufs=4) as gp, \
             tc.tile_pool(name="act", bufs=3) as ap_, \
             tc.tile_pool(name="ppool", bufs=2) as pp, \
             tc.tile_pool(name="small", bufs=4) as sp, \
             tc.tile_pool(name="mm", bufs=2, space="PSUM") as mmp, \
             tc.tile_pool(name="ep", bufs=2, space="PSUM") as epp, \
             tc.tile_pool(name="agg", bufs=2, space="PSUM") as agp, \
             tc.tile_pool(name="mps", bufs=2, space="PSUM") as mps:

            # ---- load weights / biases / idx arrays to SBUF ----
            def wload(apx):
                nm = apx.tensor.name + "_s"
                t = wp.tile(list(apx.shape), apx.dtype, tag=nm, name=nm)
                nc.sync.dma_start(out=t[:], in_=apx[:])
                return t

            Wne1_s, bne1_s = wload(Wne1), wload(bne1)
            Wne2_s, bne2r_s = wload(Wne2), wload(bne2r)
            Wee1_s, bee1_s = wload(Wee1), wload(bee1)
            Wee2_s, bee2_s = wload(Wee2), wload(bee2)
            We1_s, be1_s = wload(We1), wload(be1)
            We2_s, be2_s = wload(We2), wload(be2)
            Wn1_s, bn1r_s = wload(Wn1), wload(bn1r)
            Wu_s, bur_s, bu_s = wload(Wu), wload(bur), wload(bu)
            Wnc1_s, bnc1_s = wload(Wnc1), wload(bnc1)
            Wnc2_s, bnc2_s = wload(Wnc2), wload(bnc2)
            Wec1_s, bec1_s = wload(Wec1), wload(bec1)
            Wec2_s, bec2_s = wload(Wec2), wload(bec2)
            idxi_s, idxj_s = wload(idx_i), wload(idx_j)
            ones_s = wp.tile([1, P], BF)
            nc.gpsimd.memset(ones_s[:], 1.0)

            # ---- phase B: node embedding nf0 (all nodes, replicated) ----
            for b0 in range(0, N, TOKB):
                S = min(TOKB, N - b0)
                xb = ap_.tile([P, TOKB], BF, tag="xb")
                nc.sync.dma_start(out=xb[:, :S], in_=x_fm[:, b0:b0 + S])
                h_ps = mmp.tile([P, TOKB], F32, space="PSUM", tag="h")
                nc.tensor.matmul(out=h_ps[:, :S], lhsT=Wne1_s[:], rhs=xb[:, :S],
                                 start=True, stop=True)
                h_sb = ap_.tile([P, TOKB], BF, tag="hsb")
                nc.scalar.activation(out=h_sb[:, :S], in_=h_ps[:, :S],
                                     func=AF.Relu, bias=bne1_s[:, :1])
                for m0 in range(0, S, P):
                    ms = min(P, S - m0)
                    nf_ps = mps.tile([P, F], F32, space="PSUM", tag="sm")
                    nc.tensor.matmul(out=nf_ps[:ms, :], lhsT=h_sb[:, m0:m0 + ms],
                                     rhs=Wne2_s[:], start=True, stop=False)
                    nc.tensor.matmul(out=nf_ps[:ms, :], lhsT=ones_s[:1, :ms],
                                     rhs=bne2r_s[:1, :], start=False, stop=True)
                    nf_sb = sp.tile([P, F], BF, tag="nfsb")
                    nc.scalar.activation(out=nf_sb[:ms, :], in_=nf_ps[:ms, :],
                                         func=AF.Relu)
                    nc.sync.dma_start(out=nf0_rows[b0 + m0:b0 + m0 + ms, :],
                                      in_=nf_sb[:ms, :])
            # fill both halves of the gather table with nf0
            nc.gpsimd.dma_start(out=table[:, 0:F], in_=nf0_rows[:, :])
            nc.gpsimd.dma_start(out=table[:, F:2 * F], in_=nf0_rows[:, :])

            # ---- phase C: edge embedding ef0 ----
            for b0 in range(0, EP, TOKB):
                ea_sb = ap_.tile([64, TOKB], BF, tag="ea")
                nc.sync.dma_start(out=ea_sb[:], in_=ea_fm[:, b0:b0 + TOKB])
                h_ps = mmp.tile([P, TOKB], F32, space="PSUM", tag="h")
                nc.tensor.matmul(out=h_ps[:], lhsT=Wee1_s[:], rhs=ea_sb[:],
                                 start=True, stop=True)
                h_sb = ap_.tile([P, TOKB], BF, tag="hsb")
                nc.scalar.activation(out=h_sb[:], in_=h_ps[:],
                                     func=AF.Relu, bias=bee1_s[:, :1])
                e_ps = epp.tile([P, TOKB], F32, space="PSUM", tag="e")
                nc.tensor.matmul(out=e_ps[:], lhsT=Wee2_s[:], rhs=h_sb[:],
                                 start=True, stop=True)
                e_sb = ap_.tile([P, TOKB], BF, tag="esb")
                nc.scalar.activation(out=e_sb[:], in_=e_ps[:],
                                     func=AF.Relu, bias=bee2_s[:, :1])
                nc.sync.dma_start(out=ef0_fm[:, b0:b0 + TOKB], in_=e_sb[:])

            # ---- phase D: the two message-passing steps ----
            steps = () if stop_after < 2 else ((1,) if stop_after < 4 else (1, 2))
            for step in steps:
                agg_tiles = {}
                for c0 in range(0, EP, GCH):
                    cc = c0 // 16
                    gi = gp.tile([P, 2, GCH], BF, tag="gi")
                    nc.gpsimd.dma_gather(
                        out_ap=gi[:], in_ap=table[:],
                        idxs_ap=idxi_s[:, cc:cc + GCH // 16],
                        num_idxs=GCH, num_idxs_reg=GCH, elem_size=2 * F,
                        transpose=True)
                    gj = gp.tile([P, 2, GCH], BF, tag="gj")
                    nc.gpsimd.dma_gather(
                        out_ap=gj[:], in_ap=table[:],
                        idxs_ap=idxj_s[:, cc:cc + GCH // 16],
                        num_idxs=GCH, num_idxs_reg=GCH, elem_size=2 * F,
                        transpose=True)

                    for b0 in range(c0, c0 + GCH, TOKB):
                        lo = b0 - c0
                        e0_sb = ap_.tile([P, TOKB], BF, tag="e0")
                        nc.sync.dma_start(out=e0_sb[:], in_=ef0_fm[:, b0:b0 + TOKB])
                        if step == 1:
                            ep_sb = e0_sb
                        else:
                            ep_sb = ap_.tile([P, TOKB], BF, tag="ep")
                            nc.sync.dma_start(out=ep_sb[:],
                                              in_=ef1_fm[:, b0:b0 + TOKB])
                        chunks = [gi[:, 0, lo:lo + TOKB], gi[:, 1, lo:lo + TOKB],
                                  gj[:, 0, lo:lo + TOKB], gj[:, 1, lo:lo + TOKB],
                                  e0_sb[:], ep_sb[:]]
                        # L1: 768 -> 512
                        h1_sb = ap_.tile([P, 4, TOKB], BF, tag="h1")
                        for oc in range(4):
                            h1_ps = mmp.tile([P, TOKB], F32, space="PSUM", tag="h")
                            for kc in range(6):
                                nc.tensor.matmul(
                                    out=h1_ps[:],
                                    lhsT=We1_s[:, kc * 512 + oc * P:
                                               kc * 512 + (oc + 1) * P],
                                    rhs=chunks[kc],
                                    start=(kc == 0), stop=(kc == 5))
                            nc.scalar.activation(out=h1_sb[:, oc, :], in_=h1_ps[:],
                                                 func=AF.Relu,
                                                 bias=be1_s[:, oc:oc + 1])
                        # L2: 512 -> 128
                        e_ps = epp.tile([P, TOKB], F32, space="PSUM", tag="e")
                        for kc in range(4):
                            nc.tensor.matmul(out=e_ps[:],
                                             lhsT=We2_s[:, kc * F:(kc + 1) * F],
                                             rhs=h1_sb[:, kc, :],
                                             start=(kc == 0), stop=(kc == 3))
                        e_sb = ap_.tile([P, TOKB], BF, tag="esb")
                        nc.scalar.activation(out=e_sb[:], in_=e_ps[:],
                                             func=AF.Relu, bias=be2_s[:, :1])
                        if step == 1:
                            nc.sync.dma_start(out=ef1_fm[:, b0:b0 + TOKB],
                                              in_=e_sb[:])
                        else:
                            # fused edge classifier
                            hc_ps = mmp.tile([P, TOKB], F32, space="PSUM", tag="h")
                            nc.tensor.matmul(out=hc_ps[:], lhsT=Wec1_s[:],
                                             rhs=e_sb[:], start=True, stop=True)
                            hc_sb = ap_.tile([P, TOKB], BF, tag="hc")
                            nc.scalar.activation(out=hc_sb[:], in_=hc_ps[:],
                                                 func=AF.Relu, bias=bec1_s[:, :1])
                            pe_ps = mps.tile([1, TOKB], F32, space="PSUM", tag="sm")
                            nc.tensor.matmul(out=pe_ps[:1, :], lhsT=Wec2_s[:, :1],
                                             rhs=hc_sb[:], start=True, stop=True)
                            pe_sb = sp.tile([1, TOKB], F32, tag="pesb")
                            nc.vector.tensor_tensor(
                                out=pe_sb[:1, :], in0=pe_ps[:1, :],
                                in1=bec2_s[:1, :1].to_broadcast([1, TOKB]),
                                op=mybir.AluOpType.add)
                            nc.sync.dma_start(out=pe_o[:1, b0:b0 + TOKB],
                                              in_=pe_sb[:1, :])

                        # message layer + scatter, per 128-token tile
                        for t in range(TOKB // P):
                            tg = b0 // P + t          # global tile index
                            k, jj = tg // Tb, tg % Tb
                            s0 = t * P
                            m_ps = mps.tile([P, F], F32, space="PSUM", tag="sm")
                            mm_chunks = [gi[:, 0, lo + s0:lo + s0 + P],
                                         gi[:, 1, lo + s0:lo + s0 + P],
                                         e_sb[:, s0:s0 + P]]
                            for kc in range(3):
                                nc.tensor.matmul(out=m_ps[:],
                                                 lhsT=mm_chunks[kc],
                                                 rhs=Wn1_s[:, kc * F:(kc + 1) * F],
                                                 start=(kc == 0), stop=False)
                            nc.tensor.matmul(out=m_ps[:], lhsT=ones_s[:1, :],
                                             rhs=bn1r_s[:1, :],
                                             start=False, stop=True)
                            m_sb = sp.tile([P, F], BF, tag="msb")
                            nc.scalar.activation(out=m_sb[:], in_=m_ps[:],
                                                 func=AF.Relu)
                            if jj == 0:
                                pblk = pp.tile([P, BLK], BF, tag="pblk")
                                nc.sync.dma_start(out=pblk[:], in_=Pm[k])
                                agg_ps_new = agp.tile([P, P], F32, space="PSUM",
                                                      tag="agg", name="aggps")
                                agg_tiles[k] = (agg_ps_new, pblk)
                            agg_ps, pblk = agg_tiles[k]
                            nc.tensor.matmul(out=agg_ps[:], lhsT=m_sb[:],
                                             rhs=pblk[:, jj * P:(jj + 1) * P],
                                             start=(jj == 0), stop=(jj == Tb - 1))
                            if jj == Tb - 1:
                                aggT_sb = sp.tile([P, P], BF, tag="aggsb")
                                nc.scalar.activation(out=aggT_sb[:], in_=agg_ps[:],
                                                     func=AF.Copy)
                                if step == 1:
                                    nfr_ps = mps.tile([P, F], F32, space="PSUM",
                                                      tag="sm")
                                    nc.tensor.matmul(out=nfr_ps[:], lhsT=aggT_sb[:],
                                                     rhs=Wu_s[:], start=True,
                                                     stop=False)
                                    nc.tensor.matmul(out=nfr_ps[:],
                                                     lhsT=ones_s[:1, :],
                                                     rhs=bur_s[:1, :],
                                                     start=False, stop=True)
                                    nfr_sb = sp.tile([P, F], BF, tag="nfrsb")
                                    nc.scalar.activation(out=nfr_sb[:],
                                                         in_=nfr_ps[:],
                                                         func=AF.Relu)
                                    nc.sync.dma_start(
                                        out=nf1_rows[k * P:(k + 1) * P, :],
                                        in_=nfr_sb[:])
                                else:
                                    # feature-major update + fused node classifier
                                    nff_ps = mps.tile([P, F], F32, space="PSUM",
                                                      tag="sm")
                                    nc.tensor.matmul(out=nff_ps[:], lhsT=Wu_s[:],
                                                     rhs=aggT_sb[:], start=True,
                                                     stop=True)
                                    nff_sb = sp.tile([P, F], BF, tag="nffsb")
                                    nc.scalar.activation(out=nff_sb[:],
                                                         in_=nff_ps[:],
                                                         func=AF.Relu,
                                                         bias=bu_s[:, :1])
                                    hn_ps = mps.tile([P, F], F32, space="PSUM",
                                                     tag="sm")
                                    nc.tensor.matmul(out=hn_ps[:], lhsT=Wnc1_s[:],
                                                     rhs=nff_sb[:], start=True,
                                                     stop=True)
                                    hn_sb = sp.tile([P, F], BF, tag="hnsb")
                                    nc.scalar.activation(out=hn_sb[:],
                                                         in_=hn_ps[:],
                                                         func=AF.Relu,
                                                         bias=bnc1_s[:, :1])
                                    pn_ps = mps.tile([1, F], F32, space="PSUM",
                                                     tag="sm")
                                    nc.tensor.matmul(out=pn_ps[:1, :],
                                                     lhsT=Wnc2_s[:, :1],
                                                     rhs=hn_sb[:], start=True,
                                                     stop=True)
                                    pn_sb = sp.tile([1, F], F32, tag="pnsb")
                                    nc.vector.tensor_tensor(
                                        out=pn_sb[:1, :], in0=pn_ps[:1, :],
                                        in1=bnc2_s[:1, :1].to_broadcast([1, F]),
                                        op=mybir.AluOpType.add)
                                    nc.sync.dma_start(
                                        out=pn_o[:1, k * P:(k + 1) * P],
                                        in_=pn_sb[:1, :])
                                del agg_tiles[k]

                if step == 1 and stop_after >= 3:
                    # AllGather updated node features; refresh table second half
                    nc.gpsimd.collective_compute(
                        "AllGather", mybir.AluOpType.bypass,
                        replica_groups=[list(range(NC))],
                        ins=[nf1_rows[0:NPC, :]],
                        outs=[nf1_all[:, :]])
                    nc.gpsimd.dma_start(out=table[:, F:2 * F], in_=nf1_all[:, :])

    nc.compile()
    return nc


# --------------------------------------------------------------------------
# host-side preprocessing / sharding
# --------------------------------------------------------------------------

def _pack_weights(inputs):
    bf = ml_dtypes.bfloat16
    f32 = np.float32

    def c(a, dty=bf):
        return np.ascontiguousarray(np.asarray(a), dtype=dty)

    w = {}
    w["Wne1"] = c(inputs["Wne1"]); w["bne1"] = c(inputs["bne1"], f32).reshape(P, 1)
    w["Wne2"] = c(inputs["Wne2"]); w["bne2r"] = c(inputs["bne2"]).reshape(1, F)
    w["Wee1"] = c(inputs["Wee1"]); w["bee1"] = c(inputs["bee1"], f32).reshape(P, 1)
    w["Wee2"] = c(inputs["Wee2"]); w["bee2"] = c(inputs["bee2"], f32).reshape(P, 1)
    We1 = np.asarray(inputs["We1"], dtype=np.float32)          # [768, 512]
    w["We1"] = c(We1.reshape(6, P, 512).transpose(1, 0, 2).reshape(P, 6 * 512))
    w["be1"] = c(np.asarray(inputs["be1"], np.float32).reshape(4, P).T, f32)
    We2 = np.asarray(inputs["We2"], dtype=np.float32)          # [512, 128]
    w["We2"] = c(We2.reshape(4, P, F).transpose(1, 0, 2).reshape(P, 4 * F))
    w["be2"] = c(inputs["be2"], f32).reshape(P, 1)
    Wn1 = np.asarray(inputs["Wn1"], dtype=np.float32)          # [384, 128]
    w["Wn1"] = c(Wn1.reshape(3, P, F).transpose(1, 0, 2).reshape(P, 3 * F))
    w["bn1r"] = c(inputs["bn1"]).reshape(1, F)
    w["Wu"] = c(inputs["Wu"])
    w["bur"] = c(inputs["bu"]).reshape(1, F)
    w["bu"] = c(inputs["bu"], f32).reshape(P, 1)
    w["Wnc1"] = c(inputs["Wnc1"]); w["bnc1"] = c(inputs["bnc1"], f32).reshape(P, 1)
    w["Wnc2"] = c(inputs["Wnc2"]); w["bnc2"] = c(inputs["bnc2"], f32).reshape(1, 1)
    w["Wec1"] = c(inputs["Wec1"]); w["bec1"] = c(inputs["bec1"], f32).reshape(P, 1)
    w["Wec2"] = c(inputs["Wec2"]); w["bec2"] = c(inputs["bec2"], f32).reshape(1, 1)
    return w


def _wrap_idx(idx, EP):
    """int array [EP] -> wrapped [128, EP//16] int16 replicated across Q7 cores."""
    wi = idx.astype(np.int16).reshape(EP // 16, 16).T
    return np.ascontiguousarray(np.tile(wi, (8, 1)))


def _preprocess(inputs):
    bf = ml_dtypes.bfloat16
    x = np.asarray(inputs["x"], np.float32)
    ea = np.asarray(inputs["edge_attr"], np.float32)
    ei = np.asarray(inputs["edge_index"])
    j_src, i_tgt = ei[0].astype(np.int64), ei[1].astype(np.int64)

    core = i_tgt // NPC
    local = i_tgt - core * NPC
    blk = np.minimum(local // P, NB - 1)

    cnt = np.bincount(core * NB + blk, minlength=NC * NB).reshape(NC, NB)
    Tb = int(np.ceil(cnt.max() / P))
    if Tb % 2:
        Tb += 1
    EP = NB * Tb * P
    BLK = Tb * P

    x_fm = np.ascontiguousarray(x.T, dtype=bf)
    w = _pack_weights(inputs)

    in_maps = []
    metas = []
    order_all = np.lexsort((blk, core))       # edges grouped by (core, block)
    # split per core/block
    bounds = np.searchsorted(core[order_all], np.arange(NC + 1))
    for c in range(NC):
        ids_c = order_all[bounds[c]:bounds[c + 1]]
        blk_c = blk[ids_c]
        bb = np.searchsorted(blk_c, np.arange(NB + 1))
        slot_eid = np.full(EP, -1, np.int64)
        for k in range(NB):
            ids_k = ids_c[bb[k]:bb[k + 1]]
            slot_eid[k * BLK:k * BLK + len(ids_k)] = ids_k
        valid = slot_eid >= 0
        eid = np.where(valid, slot_eid, 0)

        ea_fm = np.zeros((64, EP), dtype=bf)
        ea_fm[:, valid] = ea[slot_eid[valid]].T.astype(bf)
        ii = np.where(valid, i_tgt[eid], 0)
        jj = np.where(valid, j_src[eid], 0)

        Pm = np.zeros((NB, P, BLK), dtype=bf)
        slots = np.arange(EP)
        vs = slots[valid]
        kk = vs // BLK
        e_in_tile = vs % P
        col = (vs % BLK) // P * P + (local[slot_eid[valid]] - kk * P)
        Pm[kk, e_in_tile, col] = 1.0

        m = dict(w)
        m.update({
            "x_fm": x_fm,
            "ea_fm": ea_fm,
            "idx_i": _wrap_idx(ii, EP),
            "idx_j": _wrap_idx(jj, EP),
            "Pm": Pm,
        })
        in_maps.append(m)
        metas.append((slot_eid, valid))
    return Tb, EP, in_maps, metas


# --------------------------------------------------------------------------
# entry point
# --------------------------------------------------------------------------

def kernel(**inputs):
    Tb, EP, in_maps, metas = _preprocess(inputs)
    if Tb not in _prog_cache:
        _prog_cache[Tb] = _build_program(Tb)
    nc = _prog_cache[Tb]

    res = run_bass_kernel_spmd(nc, in_maps, list(range(NC))).results

    preds_edge = np.zeros(E, np.float32)
    preds_node = np.zeros(N, np.float32)
    for c in range(NC):
        slot_eid, valid = metas[c]
        pe = res[c]["preds_edge"].reshape(-1)
        preds_edge[slot_eid[valid]] = pe[valid]
        pn = res[c]["preds_node"].reshape(-1)
        preds_node[c * NPC:(c + 1) * NPC] = pn[:NPC]
    return preds_edge, preds_node
